# revision 55
# baseline (speedup 1.0000x reference)
"""ConformerBlock Trainium2 kernel (Bass/Tile), 8-core SPMD.

Sharding: core c handles batch b=c//2, sequence half c%2.  Half-1 cores
receive the sequence REVERSED (and reversed conv taps) so that every core's
program is identical: own tokens are positions [0,1024), the query/conv
window is positions [0,1152), conv zero-padding is on the left edge.
Attention keys are order-invariant; the depthwise conv commutes with
reversal when taps are reversed; everything else is per-token.

ffn1 + qkv run redundantly over the full 2048-token batch so attention K/V
need no cross-core communication.

Numerics: BitLinear matmuls are exact integer bf16 matmuls (ternary bf16
weights, int8-valued bf16 activations, fp32 PSUM accumulate), descaled per
token.  MHA matmuls and the depthwise conv run in fp16.  Rounding uses the
+/-1.5*2^23 magic trick (round-to-nearest-even, matches jnp.round).

Engine split: PE does matmuls, transposes and the depthwise conv (diagonal
weights); ACT does sin/square/sigmoid/exp and PSUM evacuations; DVE does
reduces, quant scaling and PSUM-coupled element-wise; GpSimd does the
SBUF-only magic-subtract casts.
"""

from contextlib import ExitStack

import ml_dtypes
import numpy as np

import concourse.bass as bass
import concourse.mybir as mybir
import concourse.tile as tile
from concourse.bass import ts
from concourse.masks import make_identity
import json as _json


def _fix_bir(nc):
    """This container's walrus allows at most ONE sem wait per instruction.
    Hoist surplus waits: for engine instructions onto injected same-engine
    NoOps; for DMACopies onto prepended 1-element dummy copies on the same
    queue (ring order gates the real transfer, identical semantics)."""
    orig = nc.to_json_bytes

    def patched():
        import copy as _copy
        data = _json.loads(orig())
        used = set()
        for fn in data["functions"]:
            for bb in fn["blocks"]:
                for ins in bb["instructions"]:
                    si2 = ins.get("sync_info") or {}
                    for w in (si2.get("on_wait") or []):
                        used.add(w.get("id", 0))
                    for u in (si2.get("on_update") or []):
                        used.add(u.get("id", 0))
        scratch_sem = max(used) + 1 if used else 60
        k = 0
        for fn in data["functions"]:
            for bb in fn["blocks"]:
                out = []
                for ins in bb["instructions"]:
                    si = ins.get("sync_info")
                    ow = (si or {}).get("on_wait") or []
                    if len(ow) > 1:
                        if ins.get("opcode") == "DMACopy":
                            for w in ow[:-1]:
                                k += 1
                                d = _copy.deepcopy(ins)
                                d["name"] = f"W-{k}"
                                d["sync_info"] = {
                                    "on_wait": [w],
                                    "on_update": [{
                                        "ant_name": "WFIX_scratch",
                                        "id": scratch_sem,
                                        "sync_type": "semaphore",
                                        "update_mode": "sem-inc",
                                        "update_value": 1}]}
                                for ap in list(d.get("ins", [])) + list(d.get("outs", [])):
                                    if isinstance(ap, dict) and "ap" in ap:
                                        ap["ap"] = [[s, 1] for s, _ in ap["ap"]]
                                out.append(d)
                            si["on_wait"] = [ow[-1]]
                        else:
                            for w in ow[:-1]:
                                k += 1
                                nop = {"name": f"W-{k}",
                                       "engine": ins["engine"],
                                       "opcode": "NoOp", "ins": [],
                                       "outs": [],
                                       "sync_info": {"on_wait": [w]}}
                                if "debug" in ins:
                                    nop["debug"] = ins["debug"]
                                out.append(nop)
                            si["on_wait"] = [ow[-1]]
                    out.append(ins)
                bb["instructions"] = out
        return _json.dumps(data).encode()

    nc.to_json_bytes = patched
    return nc


ml_bf16 = ml_dtypes.bfloat16

P = 128
T = 2048          # tokens per batch
D = 512           # model dim
FF = 2048         # ffn hidden
H = 8             # heads
HD = 64           # head dim
KW = 31           # conv kernel
QN = 1040         # per-core query window: own 1024 + conv halo 15 (+1 pad)
OWN = 1024
NT = T // P       # 16
NQ = 9            # query tiles: 8 full + 1 partial (16 tokens)
QT8 = 16          # tokens in the 9th (partial) query tile
NO = OWN // P     # 8
ND = D // P       # 4
NF = FF // P      # 16
CIN_W = 15 + NQ * P    # conv input row: cols j <-> token j-15; DMA pads right
MAGIC = 1.5 * 2 ** 23
EPS = 1e-6

F32 = mybir.dt.float32
BF16 = mybir.dt.bfloat16
F16 = mybir.dt.float16
AX = mybir.AxisListType.X
OP = mybir.AluOpType
AF = mybir.ActivationFunctionType


# ---------------------------------------------------------------- host prep

def np_w_quant(w):
    """Host replica of reference w_quant: (ternary int8, descale 1/s)."""
    s = np.float32(1.0) / np.clip(
        np.abs(w).mean(dtype=np.float32), np.float32(1e-5), None
    ).astype(np.float32)
    q = np.clip(np.round(w * s), -1, 1).astype(np.int8)
    return q, np.float32(1.0) / s


class Spec:
    """Host-side preprocessing of all parameters (shared across cores)."""

    def __init__(self, inp):
        f32 = np.float32

        def prep(w, kt):
            # ternary, lhsT layout, partition-major [128, kt, M] so the
            # weight DMA is one fully-contiguous descriptor per partition
            q, dsc = np_w_quant(np.asarray(w, f32))
            wt = np.ascontiguousarray(q.T).astype(ml_bf16)
            wt = wt.reshape(kt, P, wt.shape[1])
            return np.ascontiguousarray(wt.transpose(1, 0, 2)), f32(dsc)

        self.w1a, self.d_w1a = prep(inp["ff1_w1"], ND)    # [4,128,2048]
        self.w2a, self.d_w2a = prep(inp["ff1_w2"], NF)    # [16,128,512]
        self.w1b, self.d_w1b = prep(inp["ff2_w1"], ND)
        self.w2b, self.d_w2b = prep(inp["ff2_w2"], NF)
        self.pw1, self.d_pw1 = prep(inp["pw1_w"], ND)     # [4,128,1024]
        self.pw2, self.d_pw2 = prep(inp["pw2_w"], ND)     # [4,128,512]

        def pmaj(a, kt, m):
            # [D_in, m] -> partition-major [128, kt, m]
            return np.ascontiguousarray(
                a.reshape(kt, P, m).transpose(1, 0, 2))

        ipw = np.asarray(inp["in_proj_w"], f32)           # [1536, 512]
        self.wqk = pmaj(np.ascontiguousarray(ipw[: 2 * D].T).astype(np.float16), ND, 2 * D)
        self.wv = pmaj(np.ascontiguousarray(ipw[2 * D:].T).astype(np.float16), ND, D)
        self.opw = pmaj(np.ascontiguousarray(np.asarray(inp["out_proj_w"], f32).T).astype(np.float16), ND, D)

        self.ipb = np.asarray(inp["in_proj_b"], f32)
        self.opb = np.asarray(inp["out_proj_b"], f32)
        self.has_ipb = bool(np.any(self.ipb != 0))
        self.has_opb = bool(np.any(self.opb != 0))

        self.nw = {}
        self.has_nw = {}
        for k in ("ff1_norm_w", "attn_norm_w", "conv_norm_w", "ff2_norm_w",
                  "final_norm_w"):
            w = np.asarray(inp[k], f32)
            self.nw[k] = w
            self.has_nw[k] = bool(np.any(w != 1.0))

        def snake(la, lb):
            a = np.exp(np.asarray(la, f32)).astype(f32)
            invb = (f32(1.0) / (np.exp(np.asarray(lb, f32)) + f32(1e-9))).astype(f32)
            return a, invb

        self.a1, self.ivb1 = snake(inp["ff1_a"], inp["ff1_b"])
        self.a2, self.ivb2 = snake(inp["ff2_a"], inp["ff2_b"])
        self.a3, self.ivb3 = snake(inp["snake_a"], inp["snake_b"])
        self.has_a1 = bool(np.any(self.a1 != 1.0))
        self.has_a2 = bool(np.any(self.a2 != 1.0))
        self.has_a3 = bool(np.any(self.a3 != 1.0))
        self.has_ivb1 = bool(np.any(np.abs(self.ivb1 - 1.0) > 1e-7))
        self.has_ivb2 = bool(np.any(np.abs(self.ivb2 - 1.0) > 1e-7))
        self.has_ivb3 = bool(np.any(np.abs(self.ivb3 - 1.0) > 1e-7))

        # depthwise conv folded with batchnorm:
        # y = conv(glu)*A + B,  A = g*rsqrt(v+1e-5), B = (dwb-m)*A + b
        A = (np.asarray(inp["bn_g"], f32)
             / np.sqrt(np.asarray(inp["bn_v"], f32) + f32(1e-5))).astype(f32)
        Bb = ((np.asarray(inp["dw_b"], f32) - np.asarray(inp["bn_m"], f32)) * A
              + np.asarray(inp["bn_b"], f32)).astype(f32)
        dw = np.asarray(inp["dw_w"], f32)[:, 0, :]        # [512, 31]
        self.wA = (dw * A[:, None]).astype(f32)           # [512, 31]
        self.convB = Bb.reshape(ND, P)                    # [4, 128]


def np_wdiag(wA):
    """[512,31] f32 -> partition-major [128, 4*31, 128] f16 diagonal taps."""
    wd = np.zeros((ND, KW, P, P), np.float32)
    idx = np.arange(P)
    wd[:, :, idx, idx] = wA.reshape(ND, P, KW).transpose(0, 2, 1)
    return np.ascontiguousarray(
        wd.transpose(2, 0, 1, 3).reshape(P, ND * KW, P)).astype(np.float16)


# ------------------------------------------------------------- device build

class Ctx:
    def __init__(self, nc, tc, st):
        self.nc, self.tc, self.st = nc, tc, st


GRP = 4  # front-group size: stats/scales batch granularity (de-barriers PE)


def _rms_rs_range(c, scratch, src, i0, g, rs):
    """rs[:, i0:i0+g] = rsqrt(mean(x^2, axis=-1) + eps) for g tiles."""
    nc = c.nc
    st6 = scratch.tile([P, GRP, 6], F32, tag="st6", name="st6")
    agg = scratch.tile([P, GRP, 2], F32, tag="st_agg", name="st_agg")
    for i in range(g):
        nc.vector.bn_stats(st6[:, i, :], src[:, i0 + i, :])
    for i in range(g):
        nc.vector.bn_aggr(agg[:, i, :], st6[:, i, :])
    t = scratch.tile([P, GRP], F32, tag="st_t", name="st_t")
    nc.vector.tensor_mul(t[:, :g], agg[:, :g, 0], agg[:, :g, 0])
    m = scratch.tile([P, GRP], F32, tag="st_m", name="st_m")
    nc.vector.scalar_tensor_tensor(out=m[:, :g], in0=agg[:, :g, 1],
                                   scalar=EPS, in1=t[:, :g],
                                   op0=OP.add, op1=OP.add)
    rc = scratch.tile([P, GRP], F32, tag="st_rc", name="st_rc")
    nc.vector.reciprocal(rc[:, :g], m[:, :g])
    nc.scalar.activation(out=rs[:, i0:i0 + g], in_=rc[:, :g], func=AF.Sqrt)


def _rms_rs(c, pool, scratch, src, ntiles):
    rs = pool.tile([P, ntiles], F32, tag="st_rs", name="st_rs")
    for g0 in range(0, ntiles, GRP):
        _rms_rs_range(c, scratch, src, g0, min(GRP, ntiles - g0), rs)
    return rs


def _absmax_batched(c, pool, src, ntiles):
    """am[P, ntiles, 1] = absmax over last axis of src [P, ntiles, D]."""
    nc = c.nc
    am = pool.tile([P, ntiles, 1], F32, tag="st_am", name="st_am")
    nc.vector.tensor_reduce(am, src[:, :ntiles, :], AX, OP.max,
                            apply_absolute_value=True)
    return am


def _quant_scales(c, pool, amn, w_dsc, extra=1.0):
    """cs = 127/max(amn,1e-5); dsc = max(amn,1e-5)*w_dsc*extra/127."""
    nc = c.nc
    n = amn.shape[-1]
    amc = pool.tile([P, n], F32, tag="st_amc", name="st_amc")
    nc.vector.tensor_scalar_max(amc, amn, 1e-5)
    rec = pool.tile([P, n], F32, tag="st_rec", name="st_rec")
    nc.vector.reciprocal(rec, amc)
    cs = pool.tile([P, n], F32, tag="st_cs", name="st_cs")
    nc.vector.tensor_scalar_mul(cs, rec, 127.0)
    dsc = pool.tile([P, n], F32, tag="st_dsc", name="st_dsc")
    nc.vector.tensor_scalar_mul(dsc, amc, float(w_dsc) * float(extra) / 127.0)
    return cs, dsc


def _nqt_range(c, pools, src, i0, g, w_dsc, xqT, dsc, csr,
               extra=1.0, skip_norm=False):
    """Front (stats+scales) + per-tile quant + DMA transpose for the tile
    range [i0, i0+g) of src.  Writes xqT/dsc/csr slices."""
    nc = c.nc
    pool, scratch = pools
    am = scratch.tile([P, GRP, 1], F32, tag="st_am", name="st_am")
    nc.vector.tensor_reduce(am[:, :g, :], src[:, i0:i0 + g, :], AX,
                            OP.max, apply_absolute_value=True)
    if skip_norm:
        amn = am[:, :g, 0]
    else:
        st6 = scratch.tile([P, GRP, 6], F32, tag="st6", name="st6")
        agg = scratch.tile([P, GRP, 2], F32, tag="st_agg", name="st_agg")
        for i in range(g):
            nc.vector.bn_stats(st6[:, i, :], src[:, i0 + i, :])
        for i in range(g):
            nc.vector.bn_aggr(agg[:, i, :], st6[:, i, :])
        t = scratch.tile([P, GRP], F32, tag="st_t", name="st_t")
        nc.vector.tensor_mul(t[:, :g], agg[:, :g, 0], agg[:, :g, 0])
        m = scratch.tile([P, GRP], F32, tag="st_m", name="st_m")
        nc.vector.scalar_tensor_tensor(out=m[:, :g], in0=agg[:, :g, 1],
                                       scalar=EPS, in1=t[:, :g],
                                       op0=OP.add, op1=OP.add)
        rc = scratch.tile([P, GRP], F32, tag="st_rc", name="st_rc")
        nc.vector.reciprocal(rc[:, :g], m[:, :g])
        rsg = scratch.tile([P, GRP], F32, tag="st_rsg", name="st_rsg")
        nc.scalar.activation(out=rsg[:, :g], in_=rc[:, :g], func=AF.Sqrt)
        # amn = absmax(x * rs) = rs * absmax(x)
        amn = scratch.tile([P, GRP], F32, tag="st_amn", name="st_amn")
        nc.vector.tensor_mul(amn[:, :g], rsg[:, :g], am[:, :g, 0])
        amn = amn[:, :g]
    amc = scratch.tile([P, GRP], F32, tag="st_amc", name="st_amc")
    nc.vector.tensor_scalar_max(amc[:, :g], amn, 1e-5)
    rec = scratch.tile([P, GRP], F32, tag="st_rec", name="st_rec")
    nc.vector.reciprocal(rec[:, :g], amc[:, :g])
    nc.vector.tensor_scalar_mul(dsc[:, i0:i0 + g], amc[:, :g],
                                float(w_dsc) * float(extra) / 127.0)
    if skip_norm:
        nc.vector.tensor_scalar_mul(csr[:, i0:i0 + g], rec[:, :g], 127.0)
    else:
        # csr = rs * cs = rs * 127 * rec
        cs = scratch.tile([P, GRP], F32, tag="st_cs", name="st_cs")
        nc.vector.tensor_scalar_mul(cs[:, :g], rec[:, :g], 127.0)
        nc.vector.tensor_mul(csr[:, i0:i0 + g], rsg[:, :g], cs[:, :g])
    for i in range(i0, i0 + g):
        r = scratch.tile([P, D], F32, tag="nq_r", name="nq_r")
        nc.vector.tensor_scalar(out=r, in0=src[:, i, :],
                                scalar1=csr[:, i:i + 1], scalar2=MAGIC,
                                op0=OP.mult, op1=OP.add)
        xq = scratch.tile([P, D], BF16, tag="nq_xq", name="nq_xq")
        nc.vector.tensor_scalar_sub(xq, r, MAGIC)
        nc.sync.dma_start_transpose(xqT[:, :, ts(i, P)], xq[:])


def _norm_quant_transpose(c, pools, src, ntiles, w_dsc, nw_b,
                          extra=1.0, skip_norm=False):
    """[rmsnorm ->] act_quant -> bf16 -> DMA-xbar transpose.
    src: [128,ntiles,512].  Returns (xqT [128,4,ntiles*128] bf16,
    dsc [128,ntiles] f32)."""
    nc = c.nc
    pool, scratch = pools
    xqT = pool.tile([P, ND, ntiles * P], BF16, tag="xqT", name="xqT")
    dsc = pool.tile([P, ntiles], F32, tag="st_dsc", name="st_dsc")
    gen = (not skip_norm) and (nw_b is not None)
    if gen:
        rs = _rms_rs(c, pool, scratch, src, ntiles)
        for i in range(ntiles):
            r = scratch.tile([P, D], F32, tag="nq_r", name="nq_r")
            xn = scratch.tile([P, D], F32, tag="nq_xn", name="nq_xn")
            nc.vector.scalar_tensor_tensor(out=xn, in0=src[:, i, :],
                                           scalar=rs[:, i:i + 1], in1=nw_b,
                                           op0=OP.mult, op1=OP.mult)
            amn1 = scratch.tile([P, 1], F32, tag="st_amn1", name="st_amn1")
            nc.vector.tensor_reduce(amn1, xn, AX, OP.max,
                                    apply_absolute_value=True)
            csl, dscl = _quant_scales(c, scratch, amn1, w_dsc, extra)
            nc.vector.tensor_copy(dsc[:, i:i + 1], dscl)
            nc.vector.tensor_scalar(out=r, in0=xn, scalar1=csl,
                                    scalar2=MAGIC, op0=OP.mult, op1=OP.add)
            xq = scratch.tile([P, D], BF16, tag="nq_xq", name="nq_xq")
            nc.vector.tensor_scalar_sub(xq, r, MAGIC)
            nc.sync.dma_start_transpose(xqT[:, :, ts(i, P)], xq[:])
        return xqT, dsc

    csr = pool.tile([P, ntiles], F32, tag="st_csr", name="st_csr")
    for g0 in range(0, ntiles, GRP):
        _nqt_range(c, pools, src, g0, min(GRP, ntiles - g0), w_dsc,
                   xqT, dsc, csr, extra, skip_norm)
    return xqT, dsc


def _ffn(c, pools, src, ntiles, w1, w2, d_w1, d_w2, a_b, ivb_b, has_a,
         has_ivb, nw_b, resid_scale, dst):
    """dst = src + resid_scale * ffn(src).  src/dst: [128,ntiles,512] f32.

    Hidden is computed in two 1024-wide halves so the PSUM hidden pool
    (psH, [P,2,512]=2 banks, bufs=2) double-buffers across halves/tiles."""
    nc = c.nc
    pool, scratch, psH, psA2 = pools
    if nw_b is not None:
        xqT, dsc1 = _norm_quant_transpose(c, (pool, scratch), src, ntiles,
                                          d_w1, nw_b)
        interleave = False
    else:
        # emit the quant front per group, interleaved with the heavy tile
        # loop below, so group 0's chain isn't scheduled behind the other
        # groups' stats and the first matmuls start ~25us earlier
        xqT = pool.tile([P, ND, ntiles * P], BF16, tag="xqT", name="xqT")
        dsc1 = pool.tile([P, ntiles], F32, tag="st_dsc", name="st_dsc")
        csr1 = pool.tile([P, ntiles], F32, tag="st_csr", name="st_csr")
        interleave = True
    for i in range(ntiles):
        if interleave and i % GRP == 0:
            _nqt_range(c, (pool, scratch), src, i, min(GRP, ntiles - i),
                       d_w1, xqT, dsc1, csr1)
        z = scratch.tile([P, FF], F32, tag="ffn_z", name="ffn_z")
        for h in range(2):
            ph = psH.tile([P, 2, D], F32, tag="ps_h", name="ps_h")
            for kt in range(ND):
                for f2 in range(2):
                    fc = h * 2 + f2
                    nc.tensor.matmul(ph[:, f2, :], xqT[:, kt, ts(i, P)],
                                     w1[:, kt, ts(fc, D)],
                                     start=(kt == 0), stop=(kt == ND - 1))
            phv = ph.rearrange("p a b -> p (a b)")  # [P, 1024]
            hw = ts(h, FF // 2)
            # snake: z = hh + invb*sin(a*hh)^2, hh = psum*dsc1
            sn = scratch.tile([P, FF // 2], F32, tag="ffn_sn", name="ffn_sn")
            if has_a:
                m = scratch.tile([P, FF // 2], F32, tag="ffn_m", name="ffn_m")
                nc.vector.scalar_tensor_tensor(out=m, in0=phv,
                                               scalar=dsc1[:, i:i + 1],
                                               in1=a_b[:, hw],
                                               op0=OP.mult, op1=OP.mult)
                nc.scalar.activation(out=sn, in_=m, func=AF.Sin)
            else:
                nc.scalar.activation(out=sn, in_=phv, func=AF.Sin,
                                     scale=dsc1[:, i:i + 1])
            u = scratch.tile([P, FF // 2], F32, tag="ffn_u", name="ffn_u")
            nc.scalar.activation(out=u, in_=sn, func=AF.Square)
            if has_ivb:
                nc.vector.tensor_mul(u, u, ivb_b[:, hw])
            nc.vector.scalar_tensor_tensor(out=z[:, hw], in0=phv,
                                           scalar=dsc1[:, i:i + 1], in1=u,
                                           op0=OP.mult, op1=OP.add)
        # act_quant(z) per token
        am2 = scratch.tile([P, 1], F32, tag="ffn_am2", name="ffn_am2")
        nc.vector.tensor_reduce(am2, z, AX, OP.max, apply_absolute_value=True)
        cs2, dsc2 = _quant_scales(c, scratch, am2, d_w2, resid_scale)
        r = scratch.tile([P, FF], F32, tag="ffn_r", name="ffn_r", bufs=1)
        nc.scalar.activation(out=r, in_=z, func=AF.Copy, scale=cs2,
                             bias=MAGIC)
        hq = scratch.tile([P, FF], BF16, tag="ffn_hq", name="ffn_hq")
        nc.vector.tensor_scalar_sub(hq, r, MAGIC)
        hqT = scratch.tile([P, NF, P], BF16, tag="ffn_hqT", name="ffn_hqT")
        nc.sync.dma_start_transpose(hqT[:], hq[:])
        p2 = psA2.tile([P, D], F32, tag="ps_o", name="ps_o")
        for kt2 in range(NF):
            nc.tensor.matmul(p2, hqT[:, kt2, :], w2[:, kt2, :],
                             start=(kt2 == 0), stop=(kt2 == NF - 1))
        nc.vector.scalar_tensor_tensor(out=dst[:, i, :], in0=p2,
                                       scalar=dsc2, in1=src[:, i, :],
                                       op0=OP.mult, op1=OP.add)


def build(spec: Spec, debug=False):
    nc = bass.Bass()
    st = spec
    if st.has_ipb:
        raise NotImplementedError("nonzero in_proj_b not supported")

    # ---- dram params
    x_d = nc.declare_dram_parameter("x", [T, D], F32, isOutput=False)
    w_names = {}
    for nm, arr, dt_ in [("w1a", st.w1a, BF16), ("w2a", st.w2a, BF16),
                         ("w1b", st.w1b, BF16), ("w2b", st.w2b, BF16),
                         ("pw1", st.pw1, BF16), ("pw2", st.pw2, BF16),
                         ("wqk", st.wqk, F16), ("wv", st.wv, F16),
                         ("opw", st.opw, F16)]:
        w_names[nm] = nc.declare_dram_parameter(nm, list(arr.shape), dt_,
                                                isOutput=False)
    wdiag_d = nc.declare_dram_parameter("wdiag", [P, ND * KW, P], F16,
                                        isOutput=False)
    convB_d = nc.declare_dram_parameter("convB", [ND, P], F32, isOutput=False)
    vec_d = {}
    for nm, need in [("a1", st.has_a1), ("ivb1", st.has_ivb1),
                     ("a2", st.has_a2), ("ivb2", st.has_ivb2),
                     ("a3", st.has_a3), ("ivb3", st.has_ivb3),
                     ("opb", st.has_opb)]:
        if need:
            n = {"a1": FF, "ivb1": FF, "a2": FF, "ivb2": FF, "a3": D,
                 "ivb3": D, "opb": D}[nm]
            vec_d[nm] = nc.declare_dram_parameter(nm, [n], F32, isOutput=False)
    nwflags = ["ff1_norm_w", "attn_norm_w", "conv_norm_w", "ff2_norm_w",
               "final_norm_w"]
    for k in nwflags:
        if st.has_nw[k]:
            vec_d[k] = nc.declare_dram_parameter(k, [D], F32, isOutput=False)

    out_d = nc.declare_dram_parameter("out", [OWN, D], F32, isOutput=True)
    if debug:
        dbg1 = nc.declare_dram_parameter("dbg_x1", [T, D], F32, isOutput=True)
        dbg2 = nc.declare_dram_parameter("dbg_x2", [QN, D], F32, isOutput=True)
        dbg3 = nc.declare_dram_parameter("dbg_x3", [OWN, D], F32, isOutput=True)

    def bcast_load(pool, dram_ap, n, tag):
        t = pool.tile([P, n], F32, tag=tag, name=tag)
        src = bass.AP(tensor=dram_ap.tensor, offset=dram_ap.offset,
                      ap=[[0, P]] + dram_ap.ap)
        nc.sync.dma_start(out=t, in_=src)
        return t

    def load_w(pool, nm, dt_, eng=None):
        arr = getattr(st, nm)  # partition-major [128, kt, M]
        t = pool.tile([P, arr.shape[1], arr.shape[2]], dt_,
                      tag=f"w_{nm}", name=f"w_{nm}")
        (eng or nc.sync).dma_start(out=t, in_=w_names[nm][:])
        return t

    with tile.TileContext(nc) as tc:
        c = Ctx(nc, tc, st)
        with ExitStack() as es:
            glob = es.enter_context(tc.tile_pool(name="glob", bufs=1))

            ones64 = glob.tile([1, HD], F16)
            nc.vector.memset(ones64, 1.0)

            # attention + conv pointwise weights preloaded early so phases
            # B and C start without waiting on their DMAs.  They go on the
            # GpSimd (SWDGE) queue so they don't delay the x-input and w1a
            # loads on the sync queue (queues complete in order).
            wqk = load_w(glob, "wqk", F16, nc.gpsimd)
            wv = load_w(glob, "wv", F16, nc.gpsimd)
            opw = load_w(glob, "opw", F16, nc.gpsimd)
            pw1 = load_w(glob, "pw1", BF16, nc.gpsimd)
            pw2 = load_w(glob, "pw2", BF16, nc.gpsimd)

            a1_b = bcast_load(glob, vec_d["a1"][:], FF, "a1b") if st.has_a1 else None
            ivb1_b = bcast_load(glob, vec_d["ivb1"][:], FF, "ivb1b") if st.has_ivb1 else None
            a2_b = bcast_load(glob, vec_d["a2"][:], FF, "a2b") if st.has_a2 else None
            ivb2_b = bcast_load(glob, vec_d["ivb2"][:], FF, "ivb2b") if st.has_ivb2 else None
            opb_b = bcast_load(glob, vec_d["opb"][:], D, "opbb") if st.has_opb else None
            nw_b = {k: (bcast_load(glob, vec_d[k][:], D, f"nw_{k}")
                        if st.has_nw[k] else None) for k in nwflags}

            # persistent residual tile for phases B..E (x2/x3/x4)
            x2 = glob.tile([P, NQ, D], F32)
            # B1 outputs live in glob so B1 can be emitted inside phase A's
            # scope (overlapping A's tail) while phase B consumes them
            xn2T = glob.tile([P, ND, T], F16)
            rs2 = glob.tile([P, NT], F32, tag="b1_rs", name="b1_rs")

            with ExitStack() as esx:
                poolX = esx.enter_context(tc.tile_pool(name="poolX", bufs=1))
                # x, then x1 after phase A (freed after B6)
                X = poolX.tile([P, NT, D], F32)
                xr = x_d[:].rearrange("(t p) d -> p t d", p=P)
                for g0 in range(0, NT, GRP):
                    nc.sync.dma_start(out=X[:, g0:g0 + GRP, :],
                                      in_=xr[:, g0:g0 + GRP, :])

                # ------------- phase A: ffn1 over full batch; X <- x1
                with tc.tile_pool(name="poolA", bufs=1) as pool, \
                     tc.tile_pool(name="scrA", bufs=2) as scratch, \
                     tc.tile_pool(name="psA", bufs=3, space="PSUM") as psA, \
                     tc.tile_pool(name="psA2", bufs=2, space="PSUM") as psA2:
                    w1 = load_w(pool, "w1a", BF16)
                    w2 = load_w(pool, "w2a", BF16)
                    _ffn(c, (pool, scratch, psA, psA2), X, NT,
                         w1, w2, st.d_w1a, st.d_w2a,
                         a1_b, ivb1_b, st.has_a1, st.has_ivb1,
                         nw_b["ff1_norm_w"], 0.5, X)
                    # B1: rmsnorm(x1) -> xn2 fp16, transposed.  Emitted here
                    # (phase A scope) so its DVE work queues right behind
                    # A's tail and overlaps the A->B transition.
                    for g0 in range(0, NT, GRP):
                        _rms_rs_range(c, scratch, X, g0, GRP, rs2)
                        for i in range(g0, g0 + GRP):
                            xn = scratch.tile([P, D], F16, tag="b1_xn",
                                              name="b1_xn")
                            if nw_b["attn_norm_w"] is None:
                                nc.vector.tensor_scalar_mul(
                                    xn, X[:, i, :], rs2[:, i:i + 1])
                            else:
                                nc.vector.scalar_tensor_tensor(
                                    out=xn, in0=X[:, i, :],
                                    scalar=rs2[:, i:i + 1],
                                    in1=nw_b["attn_norm_w"], op0=OP.mult,
                                    op1=OP.mult)
                            nc.sync.dma_start_transpose(
                                xn2T[:, :, ts(i, P)], xn[:])
                if debug:
                    d1r = dbg1[:].rearrange("(t p) d -> p t d", p=P)
                    for i in range(NT):
                        nc.sync.dma_start(out=d1r[:, i, :], in_=X[:, i, :])

                # ------------- phase B: attention -> x2 (window [0,1152))
                with tc.tile_pool(name="poolB", bufs=1) as pool, \
                     tc.tile_pool(name="scrB", bufs=2) as scratch:
                    qkT = pool.tile([P, H, T], F16)
                    V = pool.tile([P, NT, H * (HD + 1)], F16)
                    # only the per-head ones-column (z=HD) needs init; the
                    # rest is fully overwritten by the B3 copies below
                    Vv4 = V.rearrange("p t (h z) -> p t h z", z=HD + 1)
                    nc.vector.memset(Vv4[:, :, :, HD:HD + 1], 1.0)
                    # chunk-pairs sized to a [P,2,512] (2-bank) PSUM tile so
                    # psB bufs=2 double-buffers matmul against evacuation
                    PAIRS_Q = [[(0, 512), (512, 512)], [(1024, QT8)]]
                    PAIRS_K = [[(0, 512), (512, 512)],
                               [(1024, 512), (1536, 512)]]
                    with tc.tile_pool(name="psB", bufs=2, space="PSUM") as psB, \
                         tc.tile_pool(name="psB2", bufs=2, space="PSUM") as psB2:
                        # B2: q,k feature-major; B3: v token-major + ones
                        # q (mt 0..3) only needed for the 1040-token window
                        ev = 0
                        for mt in range(H):
                            pairs = PAIRS_Q if mt < 4 else PAIRS_K
                            for pr in pairs:
                                pq = psB.tile([P, 2, D], F32, tag="ps_qk",
                                              name="ps_qk")
                                for j, (n0, nw_) in enumerate(pr):
                                    for kt in range(ND):
                                        nc.tensor.matmul(
                                            pq[:, j, :nw_],
                                            wqk[:, kt, ts(mt, P)],
                                            xn2T[:, kt, n0:n0 + nw_],
                                            start=(kt == 0),
                                            stop=(kt == ND - 1))
                                base = pr[0][0]
                                wtot = pr[-1][0] + pr[-1][1] - base
                                pqv = pq.rearrange("p a b -> p (a b)")
                                ev += 1
                                if ev % 2 == 0:
                                    nc.vector.tensor_copy(
                                        qkT[:, mt, base:base + wtot],
                                        pqv[:, :wtot])
                                else:
                                    nc.scalar.activation(
                                        out=qkT[:, mt, base:base + wtot],
                                        in_=pqv[:, :wtot], func=AF.Copy)
                        for i in range(NT):
                            pv = psB2.tile([P, D], F32, tag="ps_v", name="ps_v")
                            for kt in range(ND):
                                nc.tensor.matmul(pv, xn2T[:, kt, ts(i, P)],
                                                 wv[:, kt, :],
                                                 start=(kt == 0),
                                                 stop=(kt == ND - 1))
                            vv = V[:, i, :].rearrange("p (h z) -> p h z",
                                                      z=HD + 1)
                            if i % 2 == 0:
                                nc.vector.tensor_copy(
                                    vv[:, :, 0:HD],
                                    pv.rearrange("p (h z) -> p h z", z=HD))
                            else:
                                nc.scalar.activation(
                                    out=vv[:, :, 0:HD],
                                    in_=pv.rearrange("p (h z) -> p h z", z=HD),
                                    func=AF.Copy)

                    # B4: attention per HEAD-PAIR over query window [0, QN).
                    # The two heads of a pair sit at partition offsets 0/64 of
                    # the same qkT tiles, so their score matmuls target
                    # disjoint 64-row groups and run CONCURRENTLY on the PE
                    # array (row-group tiling) -- 2x score throughput.
                    On = pool.tile([P, ND, QN], F16)
                    QCH = [(0, 512), (512, 512), (1024, QT8)]
                    with tc.tile_pool(name="psS", bufs=2, space="PSUM") as psS, \
                         tc.tile_pool(name="psO", bufs=2, space="PSUM") as psO, \
                         tc.tile_pool(name="psR", bufs=2, space="PSUM") as psR:
                        for hp in range(H // 2):
                            kf_t, qf_t = ND + hp, hp
                            for (q0, qw) in QCH:
                                PT = scratch.tile([P, NT, 2, 512], F16,
                                                  tag="PT", name="PT", bufs=1)
                                for kt in range(NT):
                                    ps = psS.tile([P, 2, 512], F32, tag="ps_s",
                                                  name="ps_s")
                                    for hh in range(2):
                                        o = hh * HD
                                        nc.tensor.matmul(
                                            ps[:, hh, :qw],
                                            qkT[o:o + HD, kf_t, ts(kt, P)],
                                            qkT[o:o + HD, qf_t, q0:q0 + qw],
                                            start=True, stop=True)
                                    nc.scalar.activation(
                                        out=PT[:, kt, :, :qw],
                                        in_=ps[:, :, :qw], func=AF.Exp,
                                        scale=0.125)
                                for hh in range(2):
                                    h = 2 * hp + hh
                                    qf_o = hh * HD
                                    po = psO.tile([HD + 1, 512], F32,
                                                  tag="ps_o", name="ps_o")
                                    for kt in range(NT):
                                        nc.tensor.matmul(
                                            po[:, :qw],
                                            V[:, kt, h * (HD + 1):(h + 1) * (HD + 1)],
                                            PT[:, kt, hh, :qw],
                                            start=(kt == 0),
                                            stop=(kt == NT - 1))
                                    # normalize rows 0:64 by row 64
                                    rec = scratch.tile([1, 512], F16,
                                                       tag="rec", name="rec")
                                    with nc.allow_low_precision(
                                            reason="softmax denom recip f16"):
                                        nc.vector.reciprocal(
                                            rec[:, :qw], po[HD:HD + 1, :qw])
                                    rb = psR.tile([HD, 512], F32, tag="ps_rb",
                                                  name="ps_rb")
                                    nc.tensor.matmul(rb[:, :qw], ones64,
                                                     rec[:, :qw],
                                                     start=True, stop=True)
                                    rb16 = scratch.tile([HD, 512], F16,
                                                        tag="rb16", name="rb16")
                                    nc.vector.tensor_copy(rb16[:, :qw],
                                                          rb[:, :qw])
                                    nc.vector.tensor_mul(
                                        On[qf_o:qf_o + HD, qf_t, q0:q0 + qw],
                                        po[0:HD, :qw], rb16[:, :qw])

                    # B6: out-proj + residual -> x2
                    with tc.tile_pool(name="psB6", bufs=2, space="PSUM") as psB6:
                        for i in range(NQ):
                            tw = P if i < NQ - 1 else QT8
                            pp = psB6.tile([P, D], F32, tag="ps_op",
                                           name="ps_op")
                            for kt in range(ND):
                                nc.tensor.matmul(pp[:tw, :],
                                                 On[:, kt, i * P:i * P + tw],
                                                 opw[:, kt, :],
                                                 start=(kt == 0),
                                                 stop=(kt == ND - 1))
                            if opb_b is not None:
                                tmp = scratch.tile([P, D], F32, tag="b6_t",
                                                   name="b6_t")
                                nc.vector.tensor_add(tmp[:tw, :], pp[:tw, :],
                                                     opb_b[:tw, :])
                                nc.vector.tensor_add(x2[:tw, i, :],
                                                     tmp[:tw, :],
                                                     X[:tw, i, :])
                            else:
                                nc.vector.scalar_tensor_tensor(
                                    out=x2[:tw, i, :], in0=pp[:tw, :],
                                    scalar=1.0, in1=X[:tw, i, :],
                                    op0=OP.mult, op1=OP.add)
                if debug:
                    d2r = dbg2[:].rearrange("(t p) d -> p t d", p=P)
                    for i in range(NQ):
                        nc.sync.dma_start(out=d2r[:, i, :], in_=x2[:, i, :])
            # X freed here (poolX closes)

            # ffn2 weights prefetched on the idle GpSimd DMA queue while
            # phase C runs (X is freed, so the space is available)
            poolWD = es.enter_context(tc.tile_pool(name="poolWD", bufs=1))
            w1b = load_w(poolWD, "w1b", BF16, nc.gpsimd)
            w2b = load_w(poolWD, "w2b", BF16, nc.gpsimd)

            # ---------------- phase C: conv branch; x2[:, :8] <- x3
            with tc.tile_pool(name="poolC", bufs=1) as pool, \
                 tc.tile_pool(name="scrC", bufs=2) as scratch:
                wd = pool.tile([P, ND * KW, P], F16)
                nc.gpsimd.dma_start(out=wd, in_=wdiag_d[:])
                convB_sb = pool.tile([P, ND], F32)
                nc.gpsimd.dma_start(out=convB_sb,
                                    in_=convB_d[:].rearrange("a p -> p a"))
                a3_sb = None
                ivb3_sb = None
                if st.has_a3:
                    a3_sb = pool.tile([P, ND], F32)
                    nc.sync.dma_start(
                        out=a3_sb,
                        in_=vec_d["a3"][:].rearrange("(a p) -> p a", p=P))
                if st.has_ivb3:
                    ivb3_sb = pool.tile([P, ND], F32)
                    nc.sync.dma_start(
                        out=ivb3_sb,
                        in_=vec_d["ivb3"][:].rearrange("(a p) -> p a", p=P))

                cin = pool.tile([P, ND, CIN_W], F16)
                # only the left zero-pad strip needs init; cols [15, 1167)
                # are written by the GLU transpose DMAs, and the conv never
                # reads past col 1053 (garbage beyond it is harmless)
                nc.vector.memset(cin[:, :, 0:15], 0.0)

                with tc.tile_pool(name="psC1", bufs=2, space="PSUM") as psC1, \
                     tc.tile_pool(name="psC", bufs=2, space="PSUM") as psC, \
                     tc.tile_pool(name="psC2", bufs=2, space="PSUM") as psC2:
                    if nw_b["conv_norm_w"] is not None:
                        xq3T, dsc3 = _norm_quant_transpose(
                            c, (pool, scratch), x2, NQ, st.d_pw1,
                            nw_b["conv_norm_w"])
                        il3 = False
                    else:
                        xq3T = pool.tile([P, ND, NQ * P], BF16, tag="xqT",
                                         name="xqT")
                        dsc3 = pool.tile([P, NQ], F32, tag="st_dsc",
                                         name="st_dsc")
                        csr3 = pool.tile([P, NQ], F32, tag="st_csr",
                                         name="st_csr")
                        il3 = True
                    for i in range(NQ):
                        if il3 and i % GRP == 0:
                            _nqt_range(c, (pool, scratch), x2, i,
                                       min(GRP, NQ - i), st.d_pw1,
                                       xq3T, dsc3, csr3)
                        pp = psC1.tile([P, 2, D], F32, tag="ps_pw1",
                                       name="ps_pw1")
                        for g in range(2):
                            for kt in range(ND):
                                nc.tensor.matmul(
                                    pp[:, g, :], xq3T[:, kt, ts(i, P)],
                                    pw1[:, kt, ts(g, D)],
                                    start=(kt == 0), stop=(kt == ND - 1))
                        sig = scratch.tile([P, D], F32, tag="c_sig",
                                           name="c_sig")
                        nc.scalar.activation(out=sig, in_=pp[:, 1, :],
                                             func=AF.Sigmoid,
                                             scale=dsc3[:, i:i + 1])
                        glu = scratch.tile([P, D], F16, tag="c_glu",
                                           name="c_glu")
                        nc.vector.scalar_tensor_tensor(
                            out=glu, in0=pp[:, 0, :],
                            scalar=dsc3[:, i:i + 1], in1=sig, op0=OP.mult,
                            op1=OP.mult)
                        j0 = 15 + i * P
                        nc.sync.dma_start_transpose(
                            cin[:, :, j0:j0 + P], glu[:])
                    # depthwise conv as 31 accumulated diagonal matmuls,
                    # then folded-bn bias + snake, all feature-major
                    z_fm = pool.tile([P, ND, OWN], F16)
                    for ct in range(ND):
                        for half in range(2):
                            t0 = half * 512
                            pcv = psC.tile([P, 512], F32, tag="ps_cv",
                                           name="ps_cv")
                            for k in range(KW):
                                nc.tensor.matmul(
                                    pcv, wd[:, ct * KW + k, :],
                                    cin[:, ct, t0 + k:t0 + k + 512],
                                    start=(k == 0), stop=(k == KW - 1))
                            # y = pcv + B;  z = y + ivb*sin(a*y)^2
                            sn = scratch.tile([P, 512], F32, tag="c_sn",
                                              name="c_sn")
                            if a3_sb is not None:
                                ya = scratch.tile([P, 512], F32, tag="c_ya",
                                                  name="c_ya")
                                nc.vector.tensor_scalar(
                                    out=ya, in0=pcv,
                                    scalar1=convB_sb[:, ct:ct + 1],
                                    scalar2=a3_sb[:, ct:ct + 1],
                                    op0=OP.add, op1=OP.mult)
                                nc.scalar.activation(out=sn, in_=ya,
                                                     func=AF.Sin)
                            else:
                                nc.scalar.activation(
                                    out=sn, in_=pcv, func=AF.Sin,
                                    bias=convB_sb[:, ct:ct + 1])
                            u = scratch.tile([P, 512], F32, tag="c_u",
                                             name="c_u")
                            nc.scalar.activation(out=u, in_=sn,
                                                 func=AF.Square)
                            if ivb3_sb is not None:
                                nc.vector.tensor_scalar_mul(
                                    u, u, ivb3_sb[:, ct:ct + 1])
                            nc.vector.scalar_tensor_tensor(
                                out=z_fm[:, ct, t0:t0 + 512], in0=pcv,
                                scalar=convB_sb[:, ct:ct + 1], in1=u,
                                op0=OP.add, op1=OP.add)
                    # transpose back to token-major (one xbar DMA per ct)
                    z_tm = pool.tile([P, NO, D], F16)
                    for ctt in range(ND):
                        nc.sync.dma_start_transpose(
                            z_tm[:, :, ts(ctt, P)], z_fm[:, ctt, :])
                    # quant + pw2 + residual (x2[:, :8] <- x3)
                    zqT = pool.tile([P, ND, NO * P], BF16, tag="zqT",
                                    name="zqT")
                    dsc4 = pool.tile([P, NO], F32, tag="st_dsc4",
                                     name="st_dsc4")
                    csr4 = pool.tile([P, NO], F32, tag="st_csr4",
                                     name="st_csr4")
                    for i8 in range(NO):
                        if i8 % GRP == 0:
                            _nqt_range(c, (pool, scratch), z_tm, i8,
                                       min(GRP, NO - i8), st.d_pw2,
                                       zqT, dsc4, csr4, skip_norm=True)
                        p2 = psC2.tile([P, D], F32, tag="ps_pw2",
                                       name="ps_pw2")
                        for kt in range(ND):
                            nc.tensor.matmul(p2, zqT[:, kt, ts(i8, P)],
                                             pw2[:, kt, :],
                                             start=(kt == 0),
                                             stop=(kt == ND - 1))
                        nc.vector.scalar_tensor_tensor(
                            out=x2[:, i8, :], in0=p2,
                            scalar=dsc4[:, i8:i8 + 1], in1=x2[:, i8, :],
                            op0=OP.mult, op1=OP.add)
            if debug:
                d3r = dbg3[:].rearrange("(t p) d -> p t d", p=P)
                for i in range(NO):
                    nc.sync.dma_start(out=d3r[:, i, :], in_=x2[:, i, :])

            # ---------------- phase D: ffn2; x2[:, :8] <- x4
            with tc.tile_pool(name="poolD", bufs=1) as pool, \
                 tc.tile_pool(name="scrD", bufs=2) as scratch, \
                 tc.tile_pool(name="psD", bufs=3, space="PSUM") as psD, \
                 tc.tile_pool(name="psD2", bufs=2, space="PSUM") as psD2:
                _ffn(c, (pool, scratch, psD, psD2), x2, NO,
                     w1b, w2b, st.d_w1b, st.d_w2b,
                     a2_b, ivb2_b, st.has_a2, st.has_ivb2,
                     nw_b["ff2_norm_w"], 0.5, x2)

            # ---------------- phase E: final rmsnorm -> out
            with tc.tile_pool(name="poolE", bufs=1) as pool, \
                 tc.tile_pool(name="scrE", bufs=2) as scratch:
                rs5 = _rms_rs(c, pool, scratch, x2, NO)
                outr = out_d[:].rearrange("(t p) d -> p t d", p=P)
                for i in range(NO):
                    o = scratch.tile([P, D], F32, tag="e_o", name="e_o")
                    if nw_b["final_norm_w"] is None:
                        nc.vector.tensor_scalar_mul(o, x2[:, i, :],
                                                    rs5[:, i:i + 1])
                    else:
                        nc.vector.scalar_tensor_tensor(
                            out=o, in0=x2[:, i, :], scalar=rs5[:, i:i + 1],
                            in1=nw_b["final_norm_w"], op0=OP.mult,
                            op1=OP.mult)
                    nc.sync.dma_start(out=outr[:, i, :], in_=o)

    return _fix_bir(nc)


# ------------------------------------------------------------------ runner

def make_in_maps(spec: Spec, x_full):
    """x_full: [4, 2048, 512] f32.  Returns per-core input maps."""
    maps = []
    shared = {"w1a": spec.w1a, "w2a": spec.w2a, "w1b": spec.w1b,
              "w2b": spec.w2b, "pw1": spec.pw1, "pw2": spec.pw2,
              "wqk": spec.wqk, "wv": spec.wv, "opw": spec.opw}
    wd_f = np_wdiag(spec.wA)
    wd_r = np_wdiag(np.ascontiguousarray(spec.wA[:, ::-1]))
    opt = {}
    for nm, need, arr in [("a1", spec.has_a1, spec.a1),
                          ("ivb1", spec.has_ivb1, spec.ivb1),
                          ("a2", spec.has_a2, spec.a2),
                          ("ivb2", spec.has_ivb2, spec.ivb2),
                          ("a3", spec.has_a3, spec.a3),
                          ("ivb3", spec.has_ivb3, spec.ivb3),
                          ("ipb", spec.has_ipb, spec.ipb),
                          ("opb", spec.has_opb, spec.opb)]:
        if need:
            opt[nm] = arr
    for k, need in spec.has_nw.items():
        if need:
            opt[k] = spec.nw[k]
    for cid in range(8):
        b, flip = cid // 2, cid % 2
        xb = x_full[b] if not flip else np.ascontiguousarray(x_full[b][::-1])
        m = {"x": np.asarray(xb, np.float32),
             "wdiag": wd_r if flip else wd_f,
             "convB": spec.convB, **shared, **opt}
        maps.append(m)
    return maps


def assemble_out(results):
    """results: list of 8 dicts with 'out' [1024, 512]."""
    y = np.zeros((4, T, D), np.float32)
    for cid in range(8):
        b, flip = cid // 2, cid % 2
        o = results[cid]["out"]
        if flip:
            y[b, OWN:] = o[::-1]
        else:
            y[b, :OWN] = o
    return y


# ------------------------------------------------------------------ entry

def kernel(**inputs):
    """Full-input entry point: shards across 8 NeuronCores internally."""
    from concourse.bass_utils import run_bass_kernel_spmd
    spec = Spec(inputs)
    nc = build(spec, debug=False)
    in_maps = make_in_maps(spec, np.asarray(inputs["x"], np.float32))
    res = run_bass_kernel_spmd(nc, in_maps, list(range(8)))
    return assemble_out(res.results).astype(np.float32)



# revision 58
# speedup vs baseline: 1.1069x; 1.1069x over previous
"""ConformerBlock Trainium2 kernel (Bass/Tile), 8-core SPMD.

Sharding: core c handles batch b=c//2, sequence half c%2.  Half-1 cores
receive the sequence REVERSED (and reversed conv taps) so that every core's
program is identical: own tokens are positions [0,1024), the query/conv
window is positions [0,1152), conv zero-padding is on the left edge.
Attention keys are order-invariant; the depthwise conv commutes with
reversal when taps are reversed; everything else is per-token.

ffn1 + qkv run redundantly over the full 2048-token batch so attention K/V
need no cross-core communication.

Numerics: BitLinear matmuls are exact integer bf16 matmuls (ternary bf16
weights, int8-valued bf16 activations, fp32 PSUM accumulate), descaled per
token.  MHA matmuls and the depthwise conv run in fp16.  Rounding uses the
+/-1.5*2^23 magic trick (round-to-nearest-even, matches jnp.round).

Engine split: PE does matmuls, transposes and the depthwise conv (diagonal
weights); ACT does sin/square/sigmoid/exp and PSUM evacuations; DVE does
reduces, quant scaling and PSUM-coupled element-wise; GpSimd does the
SBUF-only magic-subtract casts.
"""

from contextlib import ExitStack

import ml_dtypes
import numpy as np

import concourse.bass as bass
import concourse.mybir as mybir
import concourse.tile as tile
from concourse.bass import ts
from concourse.masks import make_identity
import json as _json


def _fix_bir(nc):
    """This container's walrus allows at most ONE sem wait per instruction.
    Hoist surplus waits: for engine instructions onto injected same-engine
    NoOps; for DMACopies onto prepended 1-element dummy copies on the same
    queue (ring order gates the real transfer, identical semantics)."""
    orig = nc.to_json_bytes

    def patched():
        import copy as _copy
        data = _json.loads(orig())
        used = set()
        for fn in data["functions"]:
            for bb in fn["blocks"]:
                for ins in bb["instructions"]:
                    si2 = ins.get("sync_info") or {}
                    for w in (si2.get("on_wait") or []):
                        used.add(w.get("id", 0))
                    for u in (si2.get("on_update") or []):
                        used.add(u.get("id", 0))
        scratch_sem = max(used) + 1 if used else 60
        k = 0
        for fn in data["functions"]:
            for bb in fn["blocks"]:
                out = []
                for ins in bb["instructions"]:
                    si = ins.get("sync_info")
                    ow = (si or {}).get("on_wait") or []
                    if len(ow) > 1:
                        if ins.get("opcode") == "DMACopy":
                            for w in ow[:-1]:
                                k += 1
                                d = _copy.deepcopy(ins)
                                d["name"] = f"W-{k}"
                                d["sync_info"] = {
                                    "on_wait": [w],
                                    "on_update": [{
                                        "ant_name": "WFIX_scratch",
                                        "id": scratch_sem,
                                        "sync_type": "semaphore",
                                        "update_mode": "sem-inc",
                                        "update_value": 1}]}
                                for ap in list(d.get("ins", [])) + list(d.get("outs", [])):
                                    if isinstance(ap, dict) and "ap" in ap:
                                        ap["ap"] = [[s, 1] for s, _ in ap["ap"]]
                                out.append(d)
                            si["on_wait"] = [ow[-1]]
                        else:
                            for w in ow[:-1]:
                                k += 1
                                nop = {"name": f"W-{k}",
                                       "engine": ins["engine"],
                                       "opcode": "NoOp", "ins": [],
                                       "outs": [],
                                       "sync_info": {"on_wait": [w]}}
                                if "debug" in ins:
                                    nop["debug"] = ins["debug"]
                                out.append(nop)
                            si["on_wait"] = [ow[-1]]
                    out.append(ins)
                bb["instructions"] = out
        return _json.dumps(data).encode()

    nc.to_json_bytes = patched
    return nc


ml_bf16 = ml_dtypes.bfloat16

P = 128
T = 2048          # tokens per batch
D = 512           # model dim
FF = 2048         # ffn hidden
H = 8             # heads
HD = 64           # head dim
KW = 31           # conv kernel
QN = 1040         # per-core query window: own 1024 + conv halo 15 (+1 pad)
OWN = 1024
NT = T // P       # 16
NQ = 9            # query tiles: 8 full + 1 partial (16 tokens)
QT8 = 16          # tokens in the 9th (partial) query tile
NO = OWN // P     # 8
ND = D // P       # 4
NF = FF // P      # 16
CIN_W = 15 + NQ * P    # conv input row: cols j <-> token j-15; DMA pads right
MAGIC = 1.5 * 2 ** 23
EPS = 1e-6

F32 = mybir.dt.float32
BF16 = mybir.dt.bfloat16
F16 = mybir.dt.float16
AX = mybir.AxisListType.X
OP = mybir.AluOpType
AF = mybir.ActivationFunctionType


# ---------------------------------------------------------------- host prep

def np_w_quant(w):
    """Host replica of reference w_quant: (ternary int8, descale 1/s)."""
    s = np.float32(1.0) / np.clip(
        np.abs(w).mean(dtype=np.float32), np.float32(1e-5), None
    ).astype(np.float32)
    q = np.clip(np.round(w * s), -1, 1).astype(np.int8)
    return q, np.float32(1.0) / s


class Spec:
    """Host-side preprocessing of all parameters (shared across cores)."""

    def __init__(self, inp):
        f32 = np.float32

        def prep(w, kt):
            # ternary, lhsT layout, partition-major [128, kt, M] so the
            # weight DMA is one fully-contiguous descriptor per partition
            q, dsc = np_w_quant(np.asarray(w, f32))
            wt = np.ascontiguousarray(q.T).astype(ml_bf16)
            wt = wt.reshape(kt, P, wt.shape[1])
            return np.ascontiguousarray(wt.transpose(1, 0, 2)), f32(dsc)

        self.w1a, self.d_w1a = prep(inp["ff1_w1"], ND)    # [4,128,2048]
        self.w2a, self.d_w2a = prep(inp["ff1_w2"], NF)    # [16,128,512]
        self.w1b, self.d_w1b = prep(inp["ff2_w1"], ND)
        self.w2b, self.d_w2b = prep(inp["ff2_w2"], NF)
        self.pw1, self.d_pw1 = prep(inp["pw1_w"], ND)     # [4,128,1024]
        self.pw2, self.d_pw2 = prep(inp["pw2_w"], ND)     # [4,128,512]

        def pmaj(a, kt, m):
            # [D_in, m] -> partition-major [128, kt, m]
            return np.ascontiguousarray(
                a.reshape(kt, P, m).transpose(1, 0, 2))

        ipw = np.asarray(inp["in_proj_w"], f32)           # [1536, 512]
        self.wqk = pmaj(np.ascontiguousarray(ipw[: 2 * D].T).astype(np.float16), ND, 2 * D)
        self.wv = pmaj(np.ascontiguousarray(ipw[2 * D:].T).astype(np.float16), ND, D)
        self.opw = pmaj(np.ascontiguousarray(np.asarray(inp["out_proj_w"], f32).T).astype(np.float16), ND, D)

        self.ipb = np.asarray(inp["in_proj_b"], f32)
        self.opb = np.asarray(inp["out_proj_b"], f32)
        self.has_ipb = bool(np.any(self.ipb != 0))
        self.has_opb = bool(np.any(self.opb != 0))

        self.nw = {}
        self.has_nw = {}
        for k in ("ff1_norm_w", "attn_norm_w", "conv_norm_w", "ff2_norm_w",
                  "final_norm_w"):
            w = np.asarray(inp[k], f32)
            self.nw[k] = w
            self.has_nw[k] = bool(np.any(w != 1.0))

        def snake(la, lb):
            a = np.exp(np.asarray(la, f32)).astype(f32)
            invb = (f32(1.0) / (np.exp(np.asarray(lb, f32)) + f32(1e-9))).astype(f32)
            return a, invb

        self.a1, self.ivb1 = snake(inp["ff1_a"], inp["ff1_b"])
        self.a2, self.ivb2 = snake(inp["ff2_a"], inp["ff2_b"])
        self.a3, self.ivb3 = snake(inp["snake_a"], inp["snake_b"])
        self.has_a1 = bool(np.any(self.a1 != 1.0))
        self.has_a2 = bool(np.any(self.a2 != 1.0))
        self.has_a3 = bool(np.any(self.a3 != 1.0))
        self.has_ivb1 = bool(np.any(np.abs(self.ivb1 - 1.0) > 1e-7))
        self.has_ivb2 = bool(np.any(np.abs(self.ivb2 - 1.0) > 1e-7))
        self.has_ivb3 = bool(np.any(np.abs(self.ivb3 - 1.0) > 1e-7))

        # depthwise conv folded with batchnorm:
        # y = conv(glu)*A + B,  A = g*rsqrt(v+1e-5), B = (dwb-m)*A + b
        A = (np.asarray(inp["bn_g"], f32)
             / np.sqrt(np.asarray(inp["bn_v"], f32) + f32(1e-5))).astype(f32)
        Bb = ((np.asarray(inp["dw_b"], f32) - np.asarray(inp["bn_m"], f32)) * A
              + np.asarray(inp["bn_b"], f32)).astype(f32)
        dw = np.asarray(inp["dw_w"], f32)[:, 0, :]        # [512, 31]
        self.wA = (dw * A[:, None]).astype(f32)           # [512, 31]
        self.convB = Bb.reshape(ND, P)                    # [4, 128]


def np_wdiag(wA):
    """[512,31] f32 -> partition-major [128, 4*31, 128] f16 diagonal taps."""
    wd = np.zeros((ND, KW, P, P), np.float32)
    idx = np.arange(P)
    wd[:, :, idx, idx] = wA.reshape(ND, P, KW).transpose(0, 2, 1)
    return np.ascontiguousarray(
        wd.transpose(2, 0, 1, 3).reshape(P, ND * KW, P)).astype(np.float16)


# ------------------------------------------------------------- device build

class Ctx:
    def __init__(self, nc, tc, st):
        self.nc, self.tc, self.st = nc, tc, st


GRP = 4  # front-group size: stats/scales batch granularity (de-barriers PE)


def _rms_rs_range(c, scratch, src, i0, g, rs):
    """rs[:, i0:i0+g] = rsqrt(mean(x^2, axis=-1) + eps) for g tiles."""
    nc = c.nc
    st6 = scratch.tile([P, GRP, 6], F32, tag="st6", name="st6")
    agg = scratch.tile([P, GRP, 2], F32, tag="st_agg", name="st_agg")
    for i in range(g):
        nc.vector.bn_stats(st6[:, i, :], src[:, i0 + i, :])
    for i in range(g):
        nc.vector.bn_aggr(agg[:, i, :], st6[:, i, :])
    t = scratch.tile([P, GRP], F32, tag="st_t", name="st_t")
    nc.vector.tensor_mul(t[:, :g], agg[:, :g, 0], agg[:, :g, 0])
    m = scratch.tile([P, GRP], F32, tag="st_m", name="st_m")
    nc.vector.scalar_tensor_tensor(out=m[:, :g], in0=agg[:, :g, 1],
                                   scalar=EPS, in1=t[:, :g],
                                   op0=OP.add, op1=OP.add)
    rc = scratch.tile([P, GRP], F32, tag="st_rc", name="st_rc")
    nc.vector.reciprocal(rc[:, :g], m[:, :g])
    nc.scalar.activation(out=rs[:, i0:i0 + g], in_=rc[:, :g], func=AF.Sqrt)


def _rms_rs(c, pool, scratch, src, ntiles):
    rs = pool.tile([P, ntiles], F32, tag="st_rs", name="st_rs")
    for g0 in range(0, ntiles, GRP):
        _rms_rs_range(c, scratch, src, g0, min(GRP, ntiles - g0), rs)
    return rs


def _absmax_batched(c, pool, src, ntiles):
    """am[P, ntiles, 1] = absmax over last axis of src [P, ntiles, D]."""
    nc = c.nc
    am = pool.tile([P, ntiles, 1], F32, tag="st_am", name="st_am")
    nc.vector.tensor_reduce(am, src[:, :ntiles, :], AX, OP.max,
                            apply_absolute_value=True)
    return am


def _quant_scales(c, pool, amn, w_dsc, extra=1.0):
    """cs = 127/max(amn,1e-5); dsc = max(amn,1e-5)*w_dsc*extra/127."""
    nc = c.nc
    n = amn.shape[-1]
    amc = pool.tile([P, n], F32, tag="st_amc", name="st_amc")
    nc.vector.tensor_scalar_max(amc, amn, 1e-5)
    rec = pool.tile([P, n], F32, tag="st_rec", name="st_rec")
    nc.vector.reciprocal(rec, amc)
    cs = pool.tile([P, n], F32, tag="st_cs", name="st_cs")
    nc.vector.tensor_scalar_mul(cs, rec, 127.0)
    dsc = pool.tile([P, n], F32, tag="st_dsc", name="st_dsc")
    nc.vector.tensor_scalar_mul(dsc, amc, float(w_dsc) * float(extra) / 127.0)
    return cs, dsc


def _nqt_range(c, pools, src, i0, g, w_dsc, xqT, dsc, csr,
               extra=1.0, skip_norm=False):
    """Front (stats+scales) + per-tile quant + DMA transpose for the tile
    range [i0, i0+g) of src.  Writes xqT/dsc/csr slices."""
    nc = c.nc
    pool, scratch = pools
    am = scratch.tile([P, GRP, 1], F32, tag="st_am", name="st_am")
    nc.vector.tensor_reduce(am[:, :g, :], src[:, i0:i0 + g, :], AX,
                            OP.max, apply_absolute_value=True)
    if skip_norm:
        amn = am[:, :g, 0]
    else:
        st6 = scratch.tile([P, GRP, 6], F32, tag="st6", name="st6")
        agg = scratch.tile([P, GRP, 2], F32, tag="st_agg", name="st_agg")
        for i in range(g):
            nc.vector.bn_stats(st6[:, i, :], src[:, i0 + i, :])
        for i in range(g):
            nc.vector.bn_aggr(agg[:, i, :], st6[:, i, :])
        t = scratch.tile([P, GRP], F32, tag="st_t", name="st_t")
        nc.vector.tensor_mul(t[:, :g], agg[:, :g, 0], agg[:, :g, 0])
        m = scratch.tile([P, GRP], F32, tag="st_m", name="st_m")
        nc.vector.scalar_tensor_tensor(out=m[:, :g], in0=agg[:, :g, 1],
                                       scalar=EPS, in1=t[:, :g],
                                       op0=OP.add, op1=OP.add)
        rc = scratch.tile([P, GRP], F32, tag="st_rc", name="st_rc")
        nc.vector.reciprocal(rc[:, :g], m[:, :g])
        rsg = scratch.tile([P, GRP], F32, tag="st_rsg", name="st_rsg")
        nc.scalar.activation(out=rsg[:, :g], in_=rc[:, :g], func=AF.Sqrt)
        # amn = absmax(x * rs) = rs * absmax(x)
        amn = scratch.tile([P, GRP], F32, tag="st_amn", name="st_amn")
        nc.vector.tensor_mul(amn[:, :g], rsg[:, :g], am[:, :g, 0])
        amn = amn[:, :g]
    amc = scratch.tile([P, GRP], F32, tag="st_amc", name="st_amc")
    nc.vector.tensor_scalar_max(amc[:, :g], amn, 1e-5)
    rec = scratch.tile([P, GRP], F32, tag="st_rec", name="st_rec")
    nc.vector.reciprocal(rec[:, :g], amc[:, :g])
    nc.vector.tensor_scalar_mul(dsc[:, i0:i0 + g], amc[:, :g],
                                float(w_dsc) * float(extra) / 127.0)
    if skip_norm:
        nc.vector.tensor_scalar_mul(csr[:, i0:i0 + g], rec[:, :g], 127.0)
    else:
        # csr = rs * cs = rs * 127 * rec
        cs = scratch.tile([P, GRP], F32, tag="st_cs", name="st_cs")
        nc.vector.tensor_scalar_mul(cs[:, :g], rec[:, :g], 127.0)
        nc.vector.tensor_mul(csr[:, i0:i0 + g], rsg[:, :g], cs[:, :g])
    for i in range(i0, i0 + g):
        r = scratch.tile([P, D], F32, tag="nq_r", name="nq_r")
        nc.vector.tensor_scalar(out=r, in0=src[:, i, :],
                                scalar1=csr[:, i:i + 1], scalar2=MAGIC,
                                op0=OP.mult, op1=OP.add)
        xq = scratch.tile([P, D], BF16, tag="nq_xq", name="nq_xq")
        nc.vector.tensor_scalar_sub(xq, r, MAGIC)
        nc.sync.dma_start_transpose(xqT[:, :, ts(i, P)], xq[:])


def _norm_quant_transpose(c, pools, src, ntiles, w_dsc, nw_b,
                          extra=1.0, skip_norm=False):
    """[rmsnorm ->] act_quant -> bf16 -> DMA-xbar transpose.
    src: [128,ntiles,512].  Returns (xqT [128,4,ntiles*128] bf16,
    dsc [128,ntiles] f32)."""
    nc = c.nc
    pool, scratch = pools
    xqT = pool.tile([P, ND, ntiles * P], BF16, tag="xqT", name="xqT")
    dsc = pool.tile([P, ntiles], F32, tag="st_dsc", name="st_dsc")
    gen = (not skip_norm) and (nw_b is not None)
    if gen:
        rs = _rms_rs(c, pool, scratch, src, ntiles)
        for i in range(ntiles):
            r = scratch.tile([P, D], F32, tag="nq_r", name="nq_r")
            xn = scratch.tile([P, D], F32, tag="nq_xn", name="nq_xn")
            nc.vector.scalar_tensor_tensor(out=xn, in0=src[:, i, :],
                                           scalar=rs[:, i:i + 1], in1=nw_b,
                                           op0=OP.mult, op1=OP.mult)
            amn1 = scratch.tile([P, 1], F32, tag="st_amn1", name="st_amn1")
            nc.vector.tensor_reduce(amn1, xn, AX, OP.max,
                                    apply_absolute_value=True)
            csl, dscl = _quant_scales(c, scratch, amn1, w_dsc, extra)
            nc.vector.tensor_copy(dsc[:, i:i + 1], dscl)
            nc.vector.tensor_scalar(out=r, in0=xn, scalar1=csl,
                                    scalar2=MAGIC, op0=OP.mult, op1=OP.add)
            xq = scratch.tile([P, D], BF16, tag="nq_xq", name="nq_xq")
            nc.vector.tensor_scalar_sub(xq, r, MAGIC)
            nc.sync.dma_start_transpose(xqT[:, :, ts(i, P)], xq[:])
        return xqT, dsc

    csr = pool.tile([P, ntiles], F32, tag="st_csr", name="st_csr")
    for g0 in range(0, ntiles, GRP):
        _nqt_range(c, pools, src, g0, min(GRP, ntiles - g0), w_dsc,
                   xqT, dsc, csr, extra, skip_norm)
    return xqT, dsc


def _ffn(c, pools, src, ntiles, w1, w2, d_w1, d_w2, a_b, ivb_b, has_a,
         has_ivb, nw_b, resid_scale, dst):
    """dst = src + resid_scale * ffn(src).  src/dst: [128,ntiles,512] f32.

    Hidden is computed in two 1024-wide halves so the PSUM hidden pool
    (psH, [P,2,512]=2 banks, bufs=2) double-buffers across halves/tiles."""
    nc = c.nc
    pool, scratch, psH, psA2 = pools
    xqT, dsc1 = _norm_quant_transpose(c, (pool, scratch), src, ntiles,
                                      d_w1, nw_b)
    for i in range(ntiles):
        z = scratch.tile([P, FF], F32, tag="ffn_z", name="ffn_z")
        for h in range(2):
            ph = psH.tile([P, 2, D], F32, tag="ps_h", name="ps_h")
            for kt in range(ND):
                for f2 in range(2):
                    fc = h * 2 + f2
                    nc.tensor.matmul(ph[:, f2, :], xqT[:, kt, ts(i, P)],
                                     w1[:, kt, ts(fc, D)],
                                     start=(kt == 0), stop=(kt == ND - 1))
            phv = ph.rearrange("p a b -> p (a b)")  # [P, 1024]
            hw = ts(h, FF // 2)
            # snake: z = hh + invb*sin(a*hh)^2, hh = psum*dsc1
            sn = scratch.tile([P, FF // 2], F32, tag="ffn_sn", name="ffn_sn")
            if has_a:
                m = scratch.tile([P, FF // 2], F32, tag="ffn_m", name="ffn_m")
                nc.vector.scalar_tensor_tensor(out=m, in0=phv,
                                               scalar=dsc1[:, i:i + 1],
                                               in1=a_b[:, hw],
                                               op0=OP.mult, op1=OP.mult)
                nc.scalar.activation(out=sn, in_=m, func=AF.Sin)
            else:
                nc.scalar.activation(out=sn, in_=phv, func=AF.Sin,
                                     scale=dsc1[:, i:i + 1])
            u = scratch.tile([P, FF // 2], F32, tag="ffn_u", name="ffn_u")
            nc.scalar.activation(out=u, in_=sn, func=AF.Square)
            if has_ivb:
                nc.vector.tensor_mul(u, u, ivb_b[:, hw])
            nc.vector.scalar_tensor_tensor(out=z[:, hw], in0=phv,
                                           scalar=dsc1[:, i:i + 1], in1=u,
                                           op0=OP.mult, op1=OP.add)
        # act_quant(z) per token
        am2 = scratch.tile([P, 1], F32, tag="ffn_am2", name="ffn_am2")
        nc.vector.tensor_reduce(am2, z, AX, OP.max, apply_absolute_value=True)
        cs2, dsc2 = _quant_scales(c, scratch, am2, d_w2, resid_scale)
        r = scratch.tile([P, FF], F32, tag="ffn_r", name="ffn_r", bufs=1)
        nc.scalar.activation(out=r, in_=z, func=AF.Copy, scale=cs2,
                             bias=MAGIC)
        hq = scratch.tile([P, FF], BF16, tag="ffn_hq", name="ffn_hq")
        nc.vector.tensor_scalar_sub(hq, r, MAGIC)
        hqT = scratch.tile([P, NF, P], BF16, tag="ffn_hqT", name="ffn_hqT")
        nc.sync.dma_start_transpose(hqT[:], hq[:])
        p2 = psA2.tile([P, D], F32, tag="ps_o", name="ps_o")
        for kt2 in range(NF):
            nc.tensor.matmul(p2, hqT[:, kt2, :], w2[:, kt2, :],
                             start=(kt2 == 0), stop=(kt2 == NF - 1))
        nc.vector.scalar_tensor_tensor(out=dst[:, i, :], in0=p2,
                                       scalar=dsc2, in1=src[:, i, :],
                                       op0=OP.mult, op1=OP.add)


def build(spec: Spec, debug=False):
    nc = bass.Bass()
    st = spec
    if st.has_ipb:
        raise NotImplementedError("nonzero in_proj_b not supported")

    # ---- dram params
    x_d = nc.declare_dram_parameter("x", [T, D], F32, isOutput=False)
    w_names = {}
    for nm, arr, dt_ in [("w1a", st.w1a, BF16), ("w2a", st.w2a, BF16),
                         ("w1b", st.w1b, BF16), ("w2b", st.w2b, BF16),
                         ("pw1", st.pw1, BF16), ("pw2", st.pw2, BF16),
                         ("wqk", st.wqk, F16), ("wv", st.wv, F16),
                         ("opw", st.opw, F16)]:
        w_names[nm] = nc.declare_dram_parameter(nm, list(arr.shape), dt_,
                                                isOutput=False)
    wdiag_d = nc.declare_dram_parameter("wdiag", [P, ND * KW, P], F16,
                                        isOutput=False)
    convB_d = nc.declare_dram_parameter("convB", [ND, P], F32, isOutput=False)
    vec_d = {}
    for nm, need in [("a1", st.has_a1), ("ivb1", st.has_ivb1),
                     ("a2", st.has_a2), ("ivb2", st.has_ivb2),
                     ("a3", st.has_a3), ("ivb3", st.has_ivb3),
                     ("opb", st.has_opb)]:
        if need:
            n = {"a1": FF, "ivb1": FF, "a2": FF, "ivb2": FF, "a3": D,
                 "ivb3": D, "opb": D}[nm]
            vec_d[nm] = nc.declare_dram_parameter(nm, [n], F32, isOutput=False)
    nwflags = ["ff1_norm_w", "attn_norm_w", "conv_norm_w", "ff2_norm_w",
               "final_norm_w"]
    for k in nwflags:
        if st.has_nw[k]:
            vec_d[k] = nc.declare_dram_parameter(k, [D], F32, isOutput=False)

    out_d = nc.declare_dram_parameter("out", [OWN, D], F32, isOutput=True)
    if debug:
        dbg1 = nc.declare_dram_parameter("dbg_x1", [T, D], F32, isOutput=True)
        dbg2 = nc.declare_dram_parameter("dbg_x2", [QN, D], F32, isOutput=True)
        dbg3 = nc.declare_dram_parameter("dbg_x3", [OWN, D], F32, isOutput=True)

    def bcast_load(pool, dram_ap, n, tag):
        t = pool.tile([P, n], F32, tag=tag, name=tag)
        src = bass.AP(tensor=dram_ap.tensor, offset=dram_ap.offset,
                      ap=[[0, P]] + dram_ap.ap)
        nc.sync.dma_start(out=t, in_=src)
        return t

    def load_w(pool, nm, dt_, eng=None):
        arr = getattr(st, nm)  # partition-major [128, kt, M]
        t = pool.tile([P, arr.shape[1], arr.shape[2]], dt_,
                      tag=f"w_{nm}", name=f"w_{nm}")
        (eng or nc.sync).dma_start(out=t, in_=w_names[nm][:])
        return t

    with tile.TileContext(nc) as tc:
        c = Ctx(nc, tc, st)
        with ExitStack() as es:
            glob = es.enter_context(tc.tile_pool(name="glob", bufs=1))

            ones64 = glob.tile([1, HD], F16)
            nc.vector.memset(ones64, 1.0)

            # attention + conv pointwise weights preloaded early so phases
            # B and C start without waiting on their DMAs.  They go on the
            # GpSimd (SWDGE) queue so they don't delay the x-input and w1a
            # loads on the sync queue (queues complete in order).
            wqk = load_w(glob, "wqk", F16, nc.gpsimd)
            wv = load_w(glob, "wv", F16, nc.gpsimd)
            opw = load_w(glob, "opw", F16, nc.gpsimd)
            pw1 = load_w(glob, "pw1", BF16, nc.gpsimd)
            pw2 = load_w(glob, "pw2", BF16, nc.gpsimd)

            a1_b = bcast_load(glob, vec_d["a1"][:], FF, "a1b") if st.has_a1 else None
            ivb1_b = bcast_load(glob, vec_d["ivb1"][:], FF, "ivb1b") if st.has_ivb1 else None
            a2_b = bcast_load(glob, vec_d["a2"][:], FF, "a2b") if st.has_a2 else None
            ivb2_b = bcast_load(glob, vec_d["ivb2"][:], FF, "ivb2b") if st.has_ivb2 else None
            opb_b = bcast_load(glob, vec_d["opb"][:], D, "opbb") if st.has_opb else None
            nw_b = {k: (bcast_load(glob, vec_d[k][:], D, f"nw_{k}")
                        if st.has_nw[k] else None) for k in nwflags}

            # persistent residual tile for phases B..E (x2/x3/x4)
            x2 = glob.tile([P, NQ, D], F32)
            # B1 outputs live in glob so B1 can be emitted inside phase A's
            # scope (overlapping A's tail) while phase B consumes them
            xn2T = glob.tile([P, ND, T], F16)
            rs2 = glob.tile([P, NT], F32, tag="b1_rs", name="b1_rs")

            with ExitStack() as esx:
                poolX = esx.enter_context(tc.tile_pool(name="poolX", bufs=1))
                # x, then x1 after phase A (freed after B6)
                X = poolX.tile([P, NT, D], F32)
                xr = x_d[:].rearrange("(t p) d -> p t d", p=P)
                for g0 in range(0, NT, GRP):
                    nc.sync.dma_start(out=X[:, g0:g0 + GRP, :],
                                      in_=xr[:, g0:g0 + GRP, :])

                # ------------- phase A: ffn1 over full batch; X <- x1
                with tc.tile_pool(name="poolA", bufs=1) as pool, \
                     tc.tile_pool(name="scrA", bufs=2) as scratch, \
                     tc.tile_pool(name="psA", bufs=3, space="PSUM") as psA, \
                     tc.tile_pool(name="psA2", bufs=2, space="PSUM") as psA2:
                    w1 = load_w(pool, "w1a", BF16)
                    w2 = load_w(pool, "w2a", BF16)
                    _ffn(c, (pool, scratch, psA, psA2), X, NT,
                         w1, w2, st.d_w1a, st.d_w2a,
                         a1_b, ivb1_b, st.has_a1, st.has_ivb1,
                         nw_b["ff1_norm_w"], 0.5, X)
                    # B1: rmsnorm(x1) -> xn2 fp16, transposed.  Emitted here
                    # (phase A scope) so its DVE work queues right behind
                    # A's tail and overlaps the A->B transition.
                    for g0 in range(0, NT, GRP):
                        _rms_rs_range(c, scratch, X, g0, GRP, rs2)
                        for i in range(g0, g0 + GRP):
                            xn = scratch.tile([P, D], F16, tag="b1_xn",
                                              name="b1_xn")
                            if nw_b["attn_norm_w"] is None:
                                nc.vector.tensor_scalar_mul(
                                    xn, X[:, i, :], rs2[:, i:i + 1])
                            else:
                                nc.vector.scalar_tensor_tensor(
                                    out=xn, in0=X[:, i, :],
                                    scalar=rs2[:, i:i + 1],
                                    in1=nw_b["attn_norm_w"], op0=OP.mult,
                                    op1=OP.mult)
                            nc.sync.dma_start_transpose(
                                xn2T[:, :, ts(i, P)], xn[:])
                if debug:
                    d1r = dbg1[:].rearrange("(t p) d -> p t d", p=P)
                    for i in range(NT):
                        nc.sync.dma_start(out=d1r[:, i, :], in_=X[:, i, :])

                # ------------- phase B: attention -> x2 (window [0,1152))
                with tc.tile_pool(name="poolB", bufs=1) as pool, \
                     tc.tile_pool(name="scrB", bufs=2) as scratch:
                    qkT = pool.tile([P, H, T], F16)
                    V = pool.tile([P, NT, H * (HD + 1)], F16)
                    # only the per-head ones-column (z=HD) needs init; the
                    # rest is fully overwritten by the B3 copies below
                    Vv4 = V.rearrange("p t (h z) -> p t h z", z=HD + 1)
                    nc.vector.memset(Vv4[:, :, :, HD:HD + 1], 1.0)
                    # chunk-pairs sized to a [P,2,512] (2-bank) PSUM tile so
                    # psB bufs=2 double-buffers matmul against evacuation
                    PAIRS_Q = [[(0, 512), (512, 512)], [(1024, QT8)]]
                    PAIRS_K = [[(0, 512), (512, 512)],
                               [(1024, 512), (1536, 512)]]
                    with tc.tile_pool(name="psB", bufs=2, space="PSUM") as psB, \
                         tc.tile_pool(name="psB2", bufs=2, space="PSUM") as psB2:
                        # B2: q,k feature-major; B3: v token-major + ones
                        # q (mt 0..3) only needed for the 1040-token window
                        ev = 0
                        for mt in range(H):
                            pairs = PAIRS_Q if mt < 4 else PAIRS_K
                            for pr in pairs:
                                pq = psB.tile([P, 2, D], F32, tag="ps_qk",
                                              name="ps_qk")
                                for j, (n0, nw_) in enumerate(pr):
                                    for kt in range(ND):
                                        nc.tensor.matmul(
                                            pq[:, j, :nw_],
                                            wqk[:, kt, ts(mt, P)],
                                            xn2T[:, kt, n0:n0 + nw_],
                                            start=(kt == 0),
                                            stop=(kt == ND - 1))
                                base = pr[0][0]
                                wtot = pr[-1][0] + pr[-1][1] - base
                                pqv = pq.rearrange("p a b -> p (a b)")
                                ev += 1
                                if ev % 2 == 0:
                                    nc.vector.tensor_copy(
                                        qkT[:, mt, base:base + wtot],
                                        pqv[:, :wtot])
                                else:
                                    nc.scalar.activation(
                                        out=qkT[:, mt, base:base + wtot],
                                        in_=pqv[:, :wtot], func=AF.Copy)
                        for i in range(NT):
                            pv = psB2.tile([P, D], F32, tag="ps_v", name="ps_v")
                            for kt in range(ND):
                                nc.tensor.matmul(pv, xn2T[:, kt, ts(i, P)],
                                                 wv[:, kt, :],
                                                 start=(kt == 0),
                                                 stop=(kt == ND - 1))
                            vv = V[:, i, :].rearrange("p (h z) -> p h z",
                                                      z=HD + 1)
                            if i % 2 == 0:
                                nc.vector.tensor_copy(
                                    vv[:, :, 0:HD],
                                    pv.rearrange("p (h z) -> p h z", z=HD))
                            else:
                                nc.scalar.activation(
                                    out=vv[:, :, 0:HD],
                                    in_=pv.rearrange("p (h z) -> p h z", z=HD),
                                    func=AF.Copy)

                    # B4: attention per HEAD-PAIR over query window [0, QN).
                    # The two heads of a pair sit at partition offsets 0/64 of
                    # the same qkT tiles, so their score matmuls target
                    # disjoint 64-row groups and run CONCURRENTLY on the PE
                    # array (row-group tiling) -- 2x score throughput.
                    On = pool.tile([P, ND, QN], F16)
                    QCH = [(0, 512), (512, 512), (1024, QT8)]
                    with tc.tile_pool(name="psS", bufs=2, space="PSUM") as psS, \
                         tc.tile_pool(name="psO", bufs=2, space="PSUM") as psO, \
                         tc.tile_pool(name="psR", bufs=2, space="PSUM") as psR:
                        for hp in range(H // 2):
                            kf_t, qf_t = ND + hp, hp
                            for (q0, qw) in QCH:
                                PT = scratch.tile([P, NT, 2, 512], F16,
                                                  tag="PT", name="PT", bufs=1)
                                for kt in range(NT):
                                    ps = psS.tile([P, 2, 512], F32, tag="ps_s",
                                                  name="ps_s")
                                    for hh in range(2):
                                        o = hh * HD
                                        nc.tensor.matmul(
                                            ps[:, hh, :qw],
                                            qkT[o:o + HD, kf_t, ts(kt, P)],
                                            qkT[o:o + HD, qf_t, q0:q0 + qw],
                                            start=True, stop=True)
                                    nc.scalar.activation(
                                        out=PT[:, kt, :, :qw],
                                        in_=ps[:, :, :qw], func=AF.Exp,
                                        scale=0.125)
                                for hh in range(2):
                                    h = 2 * hp + hh
                                    qf_o = hh * HD
                                    po = psO.tile([HD + 1, 512], F32,
                                                  tag="ps_o", name="ps_o")
                                    for kt in range(NT):
                                        nc.tensor.matmul(
                                            po[:, :qw],
                                            V[:, kt, h * (HD + 1):(h + 1) * (HD + 1)],
                                            PT[:, kt, hh, :qw],
                                            start=(kt == 0),
                                            stop=(kt == NT - 1))
                                    # normalize rows 0:64 by row 64
                                    rec = scratch.tile([1, 512], F16,
                                                       tag="rec", name="rec")
                                    with nc.allow_low_precision(
                                            reason="softmax denom recip f16"):
                                        nc.vector.reciprocal(
                                            rec[:, :qw], po[HD:HD + 1, :qw])
                                    rb = psR.tile([HD, 512], F32, tag="ps_rb",
                                                  name="ps_rb")
                                    nc.tensor.matmul(rb[:, :qw], ones64,
                                                     rec[:, :qw],
                                                     start=True, stop=True)
                                    rb16 = scratch.tile([HD, 512], F16,
                                                        tag="rb16", name="rb16")
                                    nc.vector.tensor_copy(rb16[:, :qw],
                                                          rb[:, :qw])
                                    nc.vector.tensor_mul(
                                        On[qf_o:qf_o + HD, qf_t, q0:q0 + qw],
                                        po[0:HD, :qw], rb16[:, :qw])

                    # B6: out-proj + residual -> x2
                    with tc.tile_pool(name="psB6", bufs=2, space="PSUM") as psB6:
                        for i in range(NQ):
                            tw = P if i < NQ - 1 else QT8
                            pp = psB6.tile([P, D], F32, tag="ps_op",
                                           name="ps_op")
                            for kt in range(ND):
                                nc.tensor.matmul(pp[:tw, :],
                                                 On[:, kt, i * P:i * P + tw],
                                                 opw[:, kt, :],
                                                 start=(kt == 0),
                                                 stop=(kt == ND - 1))
                            if opb_b is not None:
                                tmp = scratch.tile([P, D], F32, tag="b6_t",
                                                   name="b6_t")
                                nc.vector.tensor_add(tmp[:tw, :], pp[:tw, :],
                                                     opb_b[:tw, :])
                                nc.vector.tensor_add(x2[:tw, i, :],
                                                     tmp[:tw, :],
                                                     X[:tw, i, :])
                            else:
                                nc.vector.scalar_tensor_tensor(
                                    out=x2[:tw, i, :], in0=pp[:tw, :],
                                    scalar=1.0, in1=X[:tw, i, :],
                                    op0=OP.mult, op1=OP.add)
                if debug:
                    d2r = dbg2[:].rearrange("(t p) d -> p t d", p=P)
                    for i in range(NQ):
                        nc.sync.dma_start(out=d2r[:, i, :], in_=x2[:, i, :])
            # X freed here (poolX closes)

            # ffn2 weights prefetched on the idle GpSimd DMA queue while
            # phase C runs (X is freed, so the space is available)
            poolWD = es.enter_context(tc.tile_pool(name="poolWD", bufs=1))
            w1b = load_w(poolWD, "w1b", BF16, nc.gpsimd)
            w2b = load_w(poolWD, "w2b", BF16, nc.gpsimd)

            # ---------------- phase C: conv branch; x2[:, :8] <- x3
            with tc.tile_pool(name="poolC", bufs=1) as pool, \
                 tc.tile_pool(name="scrC", bufs=2) as scratch:
                wd = pool.tile([P, ND * KW, P], F16)
                nc.gpsimd.dma_start(out=wd, in_=wdiag_d[:])
                convB_sb = pool.tile([P, ND], F32)
                nc.gpsimd.dma_start(out=convB_sb,
                                    in_=convB_d[:].rearrange("a p -> p a"))
                a3_sb = None
                ivb3_sb = None
                if st.has_a3:
                    a3_sb = pool.tile([P, ND], F32)
                    nc.sync.dma_start(
                        out=a3_sb,
                        in_=vec_d["a3"][:].rearrange("(a p) -> p a", p=P))
                if st.has_ivb3:
                    ivb3_sb = pool.tile([P, ND], F32)
                    nc.sync.dma_start(
                        out=ivb3_sb,
                        in_=vec_d["ivb3"][:].rearrange("(a p) -> p a", p=P))

                cin = pool.tile([P, ND, CIN_W], F16)
                # only the left zero-pad strip needs init; cols [15, 1167)
                # are written by the GLU transpose DMAs, and the conv never
                # reads past col 1053 (garbage beyond it is harmless)
                nc.vector.memset(cin[:, :, 0:15], 0.0)

                with tc.tile_pool(name="psC1", bufs=2, space="PSUM") as psC1, \
                     tc.tile_pool(name="psC", bufs=2, space="PSUM") as psC, \
                     tc.tile_pool(name="psC2", bufs=2, space="PSUM") as psC2:
                    xq3T, dsc3 = _norm_quant_transpose(
                        c, (pool, scratch), x2, NQ, st.d_pw1,
                        nw_b["conv_norm_w"])
                    for i in range(NQ):
                        pp = psC1.tile([P, 2, D], F32, tag="ps_pw1",
                                       name="ps_pw1")
                        for g in range(2):
                            for kt in range(ND):
                                nc.tensor.matmul(
                                    pp[:, g, :], xq3T[:, kt, ts(i, P)],
                                    pw1[:, kt, ts(g, D)],
                                    start=(kt == 0), stop=(kt == ND - 1))
                        sig = scratch.tile([P, D], F32, tag="c_sig",
                                           name="c_sig")
                        nc.scalar.activation(out=sig, in_=pp[:, 1, :],
                                             func=AF.Sigmoid,
                                             scale=dsc3[:, i:i + 1])
                        glu = scratch.tile([P, D], F16, tag="c_glu",
                                           name="c_glu")
                        nc.vector.scalar_tensor_tensor(
                            out=glu, in0=pp[:, 0, :],
                            scalar=dsc3[:, i:i + 1], in1=sig, op0=OP.mult,
                            op1=OP.mult)
                        j0 = 15 + i * P
                        nc.sync.dma_start_transpose(
                            cin[:, :, j0:j0 + P], glu[:])
                    # depthwise conv as 31 accumulated diagonal matmuls,
                    # then folded-bn bias + snake, all feature-major
                    z_fm = pool.tile([P, ND, OWN], F16)
                    for ct in range(ND):
                        for half in range(2):
                            t0 = half * 512
                            pcv = psC.tile([P, 512], F32, tag="ps_cv",
                                           name="ps_cv")
                            for k in range(KW):
                                nc.tensor.matmul(
                                    pcv, wd[:, ct * KW + k, :],
                                    cin[:, ct, t0 + k:t0 + k + 512],
                                    start=(k == 0), stop=(k == KW - 1))
                            # y = pcv + B;  z = y + ivb*sin(a*y)^2
                            sn = scratch.tile([P, 512], F32, tag="c_sn",
                                              name="c_sn")
                            if a3_sb is not None:
                                ya = scratch.tile([P, 512], F32, tag="c_ya",
                                                  name="c_ya")
                                nc.vector.tensor_scalar(
                                    out=ya, in0=pcv,
                                    scalar1=convB_sb[:, ct:ct + 1],
                                    scalar2=a3_sb[:, ct:ct + 1],
                                    op0=OP.add, op1=OP.mult)
                                nc.scalar.activation(out=sn, in_=ya,
                                                     func=AF.Sin)
                            else:
                                nc.scalar.activation(
                                    out=sn, in_=pcv, func=AF.Sin,
                                    bias=convB_sb[:, ct:ct + 1])
                            u = scratch.tile([P, 512], F32, tag="c_u",
                                             name="c_u")
                            nc.scalar.activation(out=u, in_=sn,
                                                 func=AF.Square)
                            if ivb3_sb is not None:
                                nc.vector.tensor_scalar_mul(
                                    u, u, ivb3_sb[:, ct:ct + 1])
                            nc.vector.scalar_tensor_tensor(
                                out=z_fm[:, ct, t0:t0 + 512], in0=pcv,
                                scalar=convB_sb[:, ct:ct + 1], in1=u,
                                op0=OP.add, op1=OP.add)
                    # transpose back to token-major (one xbar DMA per ct)
                    z_tm = pool.tile([P, NO, D], F16)
                    for ctt in range(ND):
                        nc.sync.dma_start_transpose(
                            z_tm[:, :, ts(ctt, P)], z_fm[:, ctt, :])
                    # quant + pw2 + residual (x2[:, :8] <- x3)
                    zqT, dsc4 = _norm_quant_transpose(
                        c, (pool, scratch), z_tm, NO, st.d_pw2, None,
                        skip_norm=True)
                    for i8 in range(NO):
                        p2 = psC2.tile([P, D], F32, tag="ps_pw2",
                                       name="ps_pw2")
                        for kt in range(ND):
                            nc.tensor.matmul(p2, zqT[:, kt, ts(i8, P)],
                                             pw2[:, kt, :],
                                             start=(kt == 0),
                                             stop=(kt == ND - 1))
                        nc.vector.scalar_tensor_tensor(
                            out=x2[:, i8, :], in0=p2,
                            scalar=dsc4[:, i8:i8 + 1], in1=x2[:, i8, :],
                            op0=OP.mult, op1=OP.add)
            if debug:
                d3r = dbg3[:].rearrange("(t p) d -> p t d", p=P)
                for i in range(NO):
                    nc.sync.dma_start(out=d3r[:, i, :], in_=x2[:, i, :])

            # ---------------- phase D: ffn2; x2[:, :8] <- x4
            with tc.tile_pool(name="poolD", bufs=1) as pool, \
                 tc.tile_pool(name="scrD", bufs=2) as scratch, \
                 tc.tile_pool(name="psD", bufs=3, space="PSUM") as psD, \
                 tc.tile_pool(name="psD2", bufs=2, space="PSUM") as psD2:
                _ffn(c, (pool, scratch, psD, psD2), x2, NO,
                     w1b, w2b, st.d_w1b, st.d_w2b,
                     a2_b, ivb2_b, st.has_a2, st.has_ivb2,
                     nw_b["ff2_norm_w"], 0.5, x2)

            # ---------------- phase E: final rmsnorm -> out
            with tc.tile_pool(name="poolE", bufs=1) as pool, \
                 tc.tile_pool(name="scrE", bufs=2) as scratch:
                rs5 = _rms_rs(c, pool, scratch, x2, NO)
                outr = out_d[:].rearrange("(t p) d -> p t d", p=P)
                for i in range(NO):
                    o = scratch.tile([P, D], F32, tag="e_o", name="e_o")
                    if nw_b["final_norm_w"] is None:
                        nc.vector.tensor_scalar_mul(o, x2[:, i, :],
                                                    rs5[:, i:i + 1])
                    else:
                        nc.vector.scalar_tensor_tensor(
                            out=o, in0=x2[:, i, :], scalar=rs5[:, i:i + 1],
                            in1=nw_b["final_norm_w"], op0=OP.mult,
                            op1=OP.mult)
                    nc.sync.dma_start(out=outr[:, i, :], in_=o)

    return _fix_bir(nc)


# ------------------------------------------------------------------ runner

def make_in_maps(spec: Spec, x_full):
    """x_full: [4, 2048, 512] f32.  Returns per-core input maps."""
    maps = []
    shared = {"w1a": spec.w1a, "w2a": spec.w2a, "w1b": spec.w1b,
              "w2b": spec.w2b, "pw1": spec.pw1, "pw2": spec.pw2,
              "wqk": spec.wqk, "wv": spec.wv, "opw": spec.opw}
    wd_f = np_wdiag(spec.wA)
    wd_r = np_wdiag(np.ascontiguousarray(spec.wA[:, ::-1]))
    opt = {}
    for nm, need, arr in [("a1", spec.has_a1, spec.a1),
                          ("ivb1", spec.has_ivb1, spec.ivb1),
                          ("a2", spec.has_a2, spec.a2),
                          ("ivb2", spec.has_ivb2, spec.ivb2),
                          ("a3", spec.has_a3, spec.a3),
                          ("ivb3", spec.has_ivb3, spec.ivb3),
                          ("ipb", spec.has_ipb, spec.ipb),
                          ("opb", spec.has_opb, spec.opb)]:
        if need:
            opt[nm] = arr
    for k, need in spec.has_nw.items():
        if need:
            opt[k] = spec.nw[k]
    for cid in range(8):
        b, flip = cid // 2, cid % 2
        xb = x_full[b] if not flip else np.ascontiguousarray(x_full[b][::-1])
        m = {"x": np.asarray(xb, np.float32),
             "wdiag": wd_r if flip else wd_f,
             "convB": spec.convB, **shared, **opt}
        maps.append(m)
    return maps


def assemble_out(results):
    """results: list of 8 dicts with 'out' [1024, 512]."""
    y = np.zeros((4, T, D), np.float32)
    for cid in range(8):
        b, flip = cid // 2, cid % 2
        o = results[cid]["out"]
        if flip:
            y[b, OWN:] = o[::-1]
        else:
            y[b, :OWN] = o
    return y


# ------------------------------------------------------------------ entry

def kernel(**inputs):
    """Full-input entry point: shards across 8 NeuronCores internally."""
    from concourse.bass_utils import run_bass_kernel_spmd
    spec = Spec(inputs)
    nc = build(spec, debug=False)
    in_maps = make_in_maps(spec, np.asarray(inputs["x"], np.float32))
    res = run_bass_kernel_spmd(nc, in_maps, list(range(8)))
    return assemble_out(res.results).astype(np.float32)

